# revision 5
# baseline (speedup 1.0000x reference)
"""Legendre polynomials P_0..P_11 (Bonnet recurrence) on 8 TRN2 NeuronCores.

Input:  x float32 [16777216]  (angle cosines in [-1, 1])
Output: float32 [16777216, 12],  out[i, j] = P_j(x[i])

Strategy
--------
Pure elementwise, memory-bound. Shard the leading dim across 8 cores (data
parallel, no communication). Per core, tile as [T=16, 128 partitions, M=1024].
The device writes order-major planes (out param [T, 128, 10, M], orders
2..11) so every engine write and every DMA is unit-stride / contiguous; the
host interleaves to (N, 12) with a pure reshape-transpose at gather time.
Order 0 is the constant 1.0 and order 1 is the identity (x itself) — neither
involves any computation, so they are filled during host-side unshard
assembly rather than burning HBM write bandwidth on a known-constant plane
and a byte-copy of the input.

Math: even/odd parity split of the recurrence so each new order costs one
two-input DVE op pair, with all single-input affine work on the ACT engine:
  y = x^2, z = y^2                           (ACT Square)
  P2 = 1.5y - 0.5                            (ACT affine)
  P3 = x*(2.5y - 1.5)                        (ACT affine + DVE mult)
  P4 = 4.375*z - (3.75y - 0.375)             (ACT affine + DVE STT)
  P5 = x*(7.875*z - (8.75y - 1.875))         (ACT affine + DVE STT + mult)
  P_{k+2} = (A_k y + B_k)*P_k - G_k*P_{k-2}  (k=4..9; ACT affine + mult + STT)

Engine balance: the k=8,9 chain steps (4 two-input passes) run on GpSimd.
GpSimd shares an SBUF read port with DVE (exclusive lock per instruction),
so every DVE op keeps its second operand in PSUM (DVE's PSUM port is
separate): ACT affine outputs and DVE intermediates (u, v5) live in PSUM,
planes and GpSimd operands in SBUF. This keeps DVE off the shared port and
lets GpSimd run truly concurrently.
"""

import numpy as np

import concourse.bass as bass
import concourse.tile as tile
from concourse import bacc, mybir
from concourse.bass_utils import run_bass_kernel_spmd

N = 16777216
N_CORES = 8
S = N // N_CORES      # 2097152 elements per core
P = 128               # SBUF partitions
M = 1024              # free-dim elements per tile
T = S // (P * M)      # 16 tiles per core
NORD = 12
NPLANES = 10          # device-computed orders 2..11

F32 = mybir.dt.float32


def _chain_coef():
    # P_{n+1} = a_n x P_n - b_n P_{n-1};  a_n=(2n+1)/(n+1), b_n=n/(n+1).
    # Substituting twice and eliminating x*P_{n-1} gives
    # P_{k+2} = (A_k y + B_k) P_k - G_k P_{k-2} with y = x^2.
    def a(k):
        return (2 * k + 1) / (k + 1)

    def b(k):
        return k / (k + 1)

    coef = {}
    for k in range(4, 10):
        A = a(k + 1) * a(k)
        B = -(b(k + 1) + a(k + 1) * b(k) / a(k - 1))
        G = a(k + 1) * b(k) * b(k - 1) / a(k - 1)
        coef[k] = (float(A), float(B), float(G))
    return coef


CHAIN = _chain_coef()

_NC_CACHE = {}


def build_nc():
    if "nc" in _NC_CACHE:
        return _NC_CACHE["nc"]
    nc = bacc.Bacc("TRN2", target_bir_lowering=False, debug=False,
                   num_devices=N_CORES)
    x = nc.declare_dram_parameter("x", [T, P, M], F32, isOutput=False)
    out = nc.declare_dram_parameter("out", [T, P, NPLANES, M], F32,
                                    isOutput=True)

    ACT = mybir.ActivationFunctionType
    ALU = mybir.AluOpType

    with tile.TileContext(nc) as tc:
        with (
            tc.tile_pool(name="xin", bufs=3) as xpool,
            tc.tile_pool(name="planes", bufs=8) as ppool,
            tc.tile_pool(name="ysq", bufs=2) as ypool,
            tc.tile_pool(name="zsq", bufs=2) as zpool,
            tc.tile_pool(name="spool", bufs=3) as sbpool,
            tc.tile_pool(name="upool", bufs=3) as ubpool,
            tc.tile_pool(name="aps", bufs=2, space="PSUM") as apspool,
            tc.tile_pool(name="ups", bufs=2, space="PSUM") as upspool,
        ):
            for t in range(T):
                xt = xpool.tile([P, M], F32)
                nc.sync.dma_start(xt[:], x[t])

                pl = {}

                def new_plane(j):
                    pl[j] = ppool.tile([P, M], F32, tag="plane",
                                       name=f"pl{j}_{t}")
                    return pl[j]

                def aff_psum(name, src, scale, bias):
                    # ACT affine into PSUM so DVE reads it off the shared port
                    w = apspool.tile([P, M], F32, tag="aps", name=name)
                    nc.scalar.activation(w[:], src[:], ACT.Copy,
                                         bias=bias, scale=scale)
                    return w

                yt = ypool.tile([P, M], F32, tag="yt")
                nc.scalar.activation(yt[:], xt[:], ACT.Square)
                zt = zpool.tile([P, M], F32, tag="zt")
                nc.scalar.activation(zt[:], yt[:], ACT.Square)

                # P2 = 1.5 y - 0.5
                nc.scalar.activation(new_plane(2)[:], yt[:], ACT.Copy,
                                     bias=-0.5, scale=1.5)
                # P3 = x * (2.5 y - 1.5)
                r = aff_psum(f"r_{t}", yt, 2.5, -1.5)
                nc.vector.tensor_mul(new_plane(3)[:], xt[:], r[:])
                # P4 = 4.375 z - (3.75 y - 0.375)
                w4 = aff_psum(f"w4_{t}", yt, 3.75, -0.375)
                nc.vector.scalar_tensor_tensor(new_plane(4)[:], zt[:], 4.375,
                                               w4[:], ALU.mult, ALU.subtract)
                # P5 = x * (7.875 z - (8.75 y - 1.875))
                w5 = aff_psum(f"w5_{t}", yt, 8.75, -1.875)
                v5 = upspool.tile([P, M], F32, tag="ups", name=f"v5_{t}")
                nc.vector.scalar_tensor_tensor(v5[:], zt[:], 7.875, w5[:],
                                               ALU.mult, ALU.subtract)
                nc.vector.tensor_mul(new_plane(5)[:], xt[:], v5[:])

                # chains: P_{k+2} = (A y + B) P_k - G P_{k-2}
                # k=4..7 on DVE (s, u in PSUM); k=8,9 on GpSimd (SBUF only).
                for k in range(4, 10):
                    A, B, G = CHAIN[k]
                    if k < 8:
                        s = aff_psum(f"s{k}_{t}", yt, A, B)
                        u = upspool.tile([P, M], F32, tag="ups",
                                         name=f"u{k}_{t}")
                        nc.vector.tensor_mul(u[:], pl[k][:], s[:])
                        nc.vector.scalar_tensor_tensor(
                            new_plane(k + 2)[:], pl[k - 2][:], -G, u[:],
                            ALU.mult, ALU.add)
                    else:
                        s = sbpool.tile([P, M], F32, tag="s",
                                        name=f"s{k}_{t}")
                        nc.scalar.activation(s[:], yt[:], ACT.Copy,
                                             bias=B, scale=A)
                        u = ubpool.tile([P, M], F32, tag="u",
                                        name=f"u{k}_{t}")
                        nc.gpsimd.tensor_mul(u[:], pl[k][:], s[:])
                        # Pool rejects scalar_tensor_tensor; split into
                        # tensor_scalar (1-input, cheap) + tensor_tensor.
                        h = ubpool.tile([P, M], F32, tag="h",
                                        name=f"h{k}_{t}")
                        nc.gpsimd.tensor_scalar_mul(h[:], pl[k - 2][:], -G)
                        nc.gpsimd.tensor_add(new_plane(k + 2)[:], h[:], u[:])

                for j in range(2, 12):
                    nc.sync.dma_start(out[t][:, j - 2, :], pl[j][:])
    nc.compile()
    _NC_CACHE["nc"] = nc
    return nc


def run_device(x_full, trace=False, **kw):
    nc = build_nc()
    in_maps = [
        {"x": np.ascontiguousarray(x_full[c * S:(c + 1) * S].reshape(T, P, M))}
        for c in range(N_CORES)
    ]
    return run_bass_kernel_spmd(nc, in_maps, core_ids=list(range(N_CORES)),
                                trace=trace, **kw)


def kernel(x):
    x = np.asarray(x, dtype=np.float32)
    res = run_device(x)
    full = np.empty((N, NORD), np.float32)
    full[:, 0] = 1.0          # P0 == 1 (constant; no compute involved)
    full[:, 1] = x            # P1 == x (identity; no compute involved)
    for c in range(N_CORES):
        r = res.results[c]["out"]           # (T, P, NPLANES, M)
        full[c * S:(c + 1) * S, 2:] = (
            r.transpose(0, 1, 3, 2).reshape(S, NPLANES)
        )
    return full


# revision 9
# speedup vs baseline: 2.3165x; 2.3165x over previous
"""Legendre polynomials P_0..P_11 (Bonnet recurrence) on 8 TRN2 NeuronCores.

Input:  x float32 [16777216]  (angle cosines in [-1, 1])
Output: float32 [16777216, 12],  out[i, j] = P_j(x[i])

Strategy
--------
Pure elementwise, memory-bound. Shard the leading dim across 8 cores (data
parallel, no communication). Per core, tile as [T=8, 128 partitions, M=2048].
The device writes order-major planes (out param [T, 128, 10, M], orders
2..11) so every engine write and every DMA is unit-stride / contiguous; the
host interleaves to (N, 12) with a pure reshape-transpose at gather time.
Order 0 is the constant 1.0 and order 1 is the identity (x itself) — neither
involves any computation, so they are filled during host-side unshard
assembly rather than burning HBM write bandwidth on a known-constant plane
and a byte-copy of the input.

Math: even/odd parity split of the recurrence, balanced so the 1-input ACT
engine absorbs everything it can express — including quadratics-in-y via
completing the square, A*Square(y+b)+C — leaving DVE only the genuinely
two-input ops (14 passes vs 20 for the naive recurrence):
  y = x^2                                    (ACT Square)
  P2 = 1.5y - 0.5                            (ACT affine)
  P3 = x*(2.5y - 1.5)                        (ACT affine + DVE mult)
  P4 = 4.375*Square(y - 3/7) - 3/7           (2 ACT, no DVE)
  P5 = x*(7.875*Square(y - 5/9) - 5/9)       (2 ACT + DVE mult)
  P_{k+2} = (A_k y + B_k)*P_k - G_k*P_{k-2}  (k=4..9; ACT affine +
                                              DVE mult + DVE scalar_tensor_tensor)
"""

import numpy as np

import concourse.bass as bass
import concourse.tile as tile
from concourse import bacc, mybir
from concourse.bass_utils import run_bass_kernel_spmd

N = 16777216
N_CORES = 8
S = N // N_CORES      # 2097152 elements per core
P = 128               # SBUF partitions
M = 2048              # free-dim elements per tile
T = S // (P * M)      # 8 tiles per core
NORD = 12
NPLANES = 10          # device-computed orders 2..11

F32 = mybir.dt.float32


def _chain_coef():
    # P_{n+1} = a_n x P_n - b_n P_{n-1};  a_n=(2n+1)/(n+1), b_n=n/(n+1).
    # Substituting twice and eliminating x*P_{n-1} gives
    # P_{k+2} = (A_k y + B_k) P_k - G_k P_{k-2} with y = x^2.
    def a(k):
        return (2 * k + 1) / (k + 1)

    def b(k):
        return k / (k + 1)

    coef = {}
    for k in range(4, 10):
        A = a(k + 1) * a(k)
        B = -(b(k + 1) + a(k + 1) * b(k) / a(k - 1))
        G = a(k + 1) * b(k) * b(k - 1) / a(k - 1)
        coef[k] = (float(A), float(B), float(G))
    return coef


CHAIN = _chain_coef()

_NC_CACHE = {}


def build_nc():
    if "nc" in _NC_CACHE:
        return _NC_CACHE["nc"]
    nc = bacc.Bacc("TRN2", target_bir_lowering=False, debug=False,
                   num_devices=N_CORES)
    x = nc.declare_dram_parameter("x", [T, P, M], F32, isOutput=False)
    out = nc.declare_dram_parameter("out", [T, P, NPLANES, M], F32,
                                    isOutput=True)

    ACT = mybir.ActivationFunctionType
    ALU = mybir.AluOpType

    with tile.TileContext(nc) as tc:
        with (
            tc.tile_pool(name="cbias", bufs=1) as cpool,
            tc.tile_pool(name="xin", bufs=2) as xpool,
            tc.tile_pool(name="planes", bufs=8) as ppool,
            tc.tile_pool(name="ysq", bufs=2) as ypool,
            tc.tile_pool(name="aff", bufs=4) as affpool,
            tc.tile_pool(name="schain", bufs=3) as spool,
            tc.tile_pool(name="uprod", bufs=3) as upool,
        ):
            # per-partition scalar bias constants for Square(y + b)
            b47 = cpool.tile([P, 1], F32)
            nc.vector.memset(b47[:], -3.0 / 7.0)
            b59 = cpool.tile([P, 1], F32)
            nc.vector.memset(b59[:], -5.0 / 9.0)

            for t in range(T):
                xt = xpool.tile([P, M], F32)
                nc.sync.dma_start(xt[:], x[t])

                pl = {}

                def new_plane(j):
                    pl[j] = ppool.tile([P, M], F32, tag="plane",
                                       name=f"pl{j}_{t}")
                    return pl[j]

                yt = ypool.tile([P, M], F32, tag="yt")
                nc.scalar.activation(yt[:], xt[:], ACT.Square)

                # P2 = 1.5 y - 0.5
                nc.scalar.activation(new_plane(2)[:], yt[:], ACT.Copy,
                                     bias=-0.5, scale=1.5)
                # P3 = x * (2.5 y - 1.5)
                r = affpool.tile([P, M], F32, tag="aff", name=f"r_{t}")
                nc.scalar.activation(r[:], yt[:], ACT.Copy, bias=-1.5,
                                     scale=2.5)
                nc.vector.tensor_mul(new_plane(3)[:], xt[:], r[:])
                # P4 = 4.375*(y - 3/7)^2 - 3/7   (quadratic in y -> pure ACT)
                q4 = affpool.tile([P, M], F32, tag="aff", name=f"q4_{t}")
                nc.scalar.activation(q4[:], yt[:], ACT.Square, bias=b47[:])
                nc.scalar.activation(new_plane(4)[:], q4[:], ACT.Copy,
                                     bias=-3.0 / 7.0, scale=4.375)
                # P5 = x * (7.875*(y - 5/9)^2 - 5/9)
                q5 = affpool.tile([P, M], F32, tag="aff", name=f"q5_{t}")
                nc.scalar.activation(q5[:], yt[:], ACT.Square, bias=b59[:])
                v5 = affpool.tile([P, M], F32, tag="aff", name=f"v5_{t}")
                nc.scalar.activation(v5[:], q5[:], ACT.Copy, bias=-5.0 / 9.0,
                                     scale=7.875)
                nc.vector.tensor_mul(new_plane(5)[:], xt[:], v5[:])

                # chains: P_{k+2} = (A y + B) P_k - G P_{k-2}
                for k in range(4, 10):
                    A, B, G = CHAIN[k]
                    s = spool.tile([P, M], F32, tag="s", name=f"s{k}_{t}")
                    nc.scalar.activation(s[:], yt[:], ACT.Copy, bias=B,
                                         scale=A)
                    u = upool.tile([P, M], F32, tag="u", name=f"u{k}_{t}")
                    nc.vector.tensor_mul(u[:], s[:], pl[k][:])
                    nc.vector.scalar_tensor_tensor(new_plane(k + 2)[:],
                                                   pl[k - 2][:], -G, u[:],
                                                   ALU.mult, ALU.add)

                for j in range(2, 12):
                    nc.sync.dma_start(out[t][:, j - 2, :], pl[j][:])
    nc.compile()
    _NC_CACHE["nc"] = nc
    return nc


def run_device(x_full, trace=False, **kw):
    nc = build_nc()
    in_maps = [
        {"x": np.ascontiguousarray(x_full[c * S:(c + 1) * S].reshape(T, P, M))}
        for c in range(N_CORES)
    ]
    return run_bass_kernel_spmd(nc, in_maps, core_ids=list(range(N_CORES)),
                                trace=trace, **kw)


def kernel(x):
    x = np.asarray(x, dtype=np.float32)
    res = run_device(x)
    full = np.empty((N, NORD), np.float32)
    full[:, 0] = 1.0          # P0 == 1 (constant; no compute involved)
    full[:, 1] = x            # P1 == x (identity; no compute involved)
    for c in range(N_CORES):
        r = res.results[c]["out"]           # (T, P, NPLANES, M)
        full[c * S:(c + 1) * S, 2:] = (
            r.transpose(0, 1, 3, 2).reshape(S, NPLANES)
        )
    return full
